# revision 1
# baseline (speedup 1.0000x reference)
"""DIoU loss (mean) on 8 Trainium2 NeuronCores via Bass/Tile.

Sharding: boxes [2e6, 4] are viewed as [128, 15625, 4] (partition-major)
and the 15625 columns are split across 8 cores (1956 cols/core, the tail
padded with identity boxes whose contribution is subtracted on the host).
Each core computes per-partition partial sums of iou and cd/diag; the
host finishes the mean in float64.

Per-box math (per axis a, with p1/p2/t1/t2 the box edges):
  d1 = p1-t1, d2 = p2-t2                      (delta quad z)
  h  = |d1|+|d2|,  g = (p2-p1)+(t2-t1)
  2u = g-h (overlap*2),  2e = g+h (enclosing extent*2),  d = d1+d2 (2*center diff)
  inter4 = relu(2u_x)*relu(2u_y) = 4*inter;  union4 = 4*(area_p+area_t) - inter4
  diag4 = (2e_x)^2+(2e_y)^2 = 4*diag;  cd4 = d_x^2+d_y^2 = 4*cd
  loss_i = 1 - inter/union + cd/diag = 1 - inter4/union4... (4x cancels)
Work is split DVE/ACT/GPSIMD to balance engine busy time while keeping
the iou critical chain on DVE/ACT (GPSIMD only feeds side branches).
"""

import numpy as np

import concourse.bass as bass
import concourse.mybir as mybir
from concourse import bacc
from concourse.tile import TileContext
from concourse.bass_utils import run_bass_kernel_spmd

N_BOXES = 2_000_000
P = 128
COLS = N_BOXES // P            # 15625
N_CORES = 8
W = 1956                       # columns per core (8*1956 = 15648 >= 15625)
NCH = 4                        # chunks per core (when no explicit list)
CHUNKS = [120, 306, 306, 306, 306, 306, 306]  # small head chunk cuts pipeline fill
PAD_BOXES = N_CORES * W * P - N_BOXES  # 2944

F32 = mybir.dt.float32
ALU = mybir.AluOpType
AF = mybir.ActivationFunctionType

_CACHE = {}


def _register_custom_ops():
    """Register fused DVE ops (idempotent); self-pin uops_sha."""
    import concourse.dve_ops as dve_ops_mod
    from concourse.dve_spec import Spec, Src0, Src1, Zero, maxx, relu, sq, lower
    from concourse.dve_ops import OPS, DveOp, has_src1
    from concourse.dve_uop import DveOpSpec

    def reg(name, spec):
        for op in OPS:
            if op.name == name:
                return op
        op = DveOp(name, spec, subdim=False, uops_sha={})
        OPS.append(op)
        row = dve_ops_mod._CUSTOM_DVE_ROW_BASE + len(OPS) - 1
        assert row < 0x20, "custom-DVE row field overflow"
        dve_ops_mod._SUB_OPCODE_FOR_NAME[name] = row
        dve_ops_mod.CUSTOM_DVE_SPECS[name] = spec
        for ver in ("v3", "v4"):
            sp = DveOpSpec(name=name, opcode=row, uops=lower(spec, ver=ver),
                           rd1_en=has_src1(spec))
            op.uops_sha[ver] = sp.sha(ver)
        return op

    abs2sum = reg("ANT_ABS2SUM", Spec(
        body=maxx(Src0, Zero - Src0) + maxx(Src1, Zero - Src1),
        reference=lambda in0, in1: np.abs(in0) + np.abs(in1)))
    relumul = reg("ANT_RELUMUL", Spec(
        body=relu(Src0) * relu(Src1),
        reference=lambda in0, in1: np.maximum(in0, 0) * np.maximum(in1, 0)))
    sq2sum = reg("ANT_SQ2SUM", Spec(
        body=sq(Src0) + sq(Src1),
        reference=lambda in0, in1: in0 * in0 + in1 * in1))
    return abs2sum, relumul, sq2sum



def _build_program(nch=NCH, bio=2, bwk=2, bsg=2, chunks=None, act_recip=False, act_rd=False, diag_dve=False, lag=1, area_dve=True, dma_e2=False, swdge_head=False, tail_gp=False):
    # ramped chunk sizes: small chunks at both ends shorten pipeline
    # fill/drain; interior chunks are large to amortize per-op overhead
    if chunks is None:
        fc = W // nch
        chunks = [fc] * nch
        chunks[-1] = W - fc * (nch - 1)
    nch = len(chunks)
    offs = [sum(chunks[:i]) for i in range(nch)]
    fcmax = max(chunks)
    nc = bacc.Bacc(None, target_bir_lowering=False)

    pred_d = nc.dram_tensor("pred", [P, W, 4], F32, kind="ExternalInput")
    targ_d = nc.dram_tensor("targ", [P, W, 4], F32, kind="ExternalInput")
    acc_d = nc.dram_tensor("acc", [P, nch], F32, kind="ExternalOutput")

    dve = nc.vector
    gp = nc.gpsimd
    ABS2SUM, RELUMUL, SQ2SUM = _register_custom_ops()

    with TileContext(nc) as tc:
        with (
            tc.tile_pool(name="io", bufs=bio) as io,
            tc.tile_pool(name="wk", bufs=bwk) as wk,
            tc.tile_pool(name="sg", bufs=bsg) as sg,
            tc.tile_pool(name="accp", bufs=1) as accp,
        ):
            acc = accp.tile([P, nch], F32)
            state = {}

            def front(i):
                fc = chunks[i]
                o0 = offs[i]
                pt = io.tile([P, fc, 4], F32, tag="pred")
                tt = io.tile([P, fc, 4], F32, tag="targ")
                eng = gp if (i == 0 and swdge_head) else nc.sync
                eng.dma_start(out=pt[:], in_=pred_d[:, o0:o0 + fc, :])
                eng.dma_start(out=tt[:], in_=targ_d[:, o0:o0 + fc, :])

                # box extents first: each needs only one of the two DMAs,
                # so GPSIMD can start before both loads complete
                ap = wk.tile([P, fc, 2], F32, tag="ap")
                gp.tensor_sub(ap[:], pt[:, :, 2:4], pt[:, :, 0:2])
                at = wk.tile([P, fc, 2], F32, tag="at")
                gp.tensor_sub(at[:], tt[:, :, 2:4], tt[:, :, 0:2])

                # delta quad: (d1x, d1y, d2x, d2y) = pred - targ
                z = wk.tile([P, fc, 4], F32, tag="z")
                dve.tensor_sub(z[:], pt[:], tt[:])

                # d = d1 + d2 = 2*(center diff), per axis
                dct = wk.tile([P, fc, 2], F32, tag="dct")
                gp.tensor_add(dct[:], z[:, :, 0:2], z[:, :, 2:4])

                g = wk.tile([P, fc, 2], F32, tag="g")
                dve.tensor_add(g[:], ap[:], at[:])
                # h = |d1| + |d2| fused on DVE
                h = wk.tile([P, fc, 2], F32, tag="h")
                dve._custom_dve(ABS2SUM, out=h[:], in0=z[:, :, 0:2], in1=z[:, :, 2:4])

                # 2*u (unclipped overlap) and 2*e (enclosing extent);
                # e2 = g + h built on the idle DMA engines (copy + accum)
                u2 = wk.tile([P, fc, 2], F32, tag="u2")
                dve.tensor_sub(u2[:], g[:], h[:])
                e2t = wk.tile([P, fc, 2], F32, tag="e2t")
                if dma_e2:
                    nc.sync.dma_start(out=e2t[:], in_=g[:])
                    gp.dma_start(out=e2t[:], in_=h[:], accum_op=ALU.add)
                else:
                    gp.tensor_add(e2t[:], g[:], h[:])

                sqe = wk.tile([P, fc, 2], F32, tag="sqe")
                nc.scalar.activation(sqe[:], e2t[:], AF.Square)
                sqd = wk.tile([P, fc, 2], F32, tag="sqd")
                nc.scalar.activation(sqd[:], dct[:], AF.Square)
                state[i] = (ap, at, u2, sqe, sqd)

            def recip(dst, src, scratch_tag):
                if act_recip:
                    t = sg.tile(list(src.shape), F32, tag=scratch_tag)
                    nc.scalar.activation(t[:], src[:], AF.Ln)
                    nc.scalar.activation(dst[:], t[:], AF.Exp, scale=-1.0)
                else:
                    dve.reciprocal_approx_fast(out=dst[:], in_=src[:])

            def back(i):
                fc = chunks[i]
                ap, at, u2, sqe, sqd = state.pop(i)
                # numerator pair IC = (inter4, -cd4); denominator pair UD =
                # (union4, diag4). One reciprocal and one accumulating stt
                # then yield sum(iou - cd/diag) directly.
                ic = sg.tile([P, fc, 2], F32, tag="ic")
                dve._custom_dve(RELUMUL, out=ic[:, :, 0],
                                in0=u2[:, :, 0], in1=u2[:, :, 1])
                last = tail_gp and i >= nch - tail_gp
                areap = sg.tile([P, fc], F32, tag="areap")
                (gp if (last or not area_dve) else dve).tensor_mul(
                    areap[:], ap[:, :, 0], ap[:, :, 1])
                areat = sg.tile([P, fc], F32, tag="areat")
                gp.tensor_mul(areat[:], at[:, :, 0], at[:, :, 1])
                asum = sg.tile([P, fc], F32, tag="asum")
                (gp if last else dve).tensor_add(asum[:], areap[:], areat[:])
                ud = sg.tile([P, fc, 2], F32, tag="ud")
                dve.scalar_tensor_tensor(
                    out=ud[:, :, 0], in0=asum[:], scalar=4.0, in1=ic[:, :, 0],
                    op0=ALU.mult, op1=ALU.subtract)
                # -cd4 = -dx^2 - dy^2
                dve.scalar_tensor_tensor(
                    out=ic[:, :, 1], in0=sqd[:, :, 0], scalar=-1.0,
                    in1=sqd[:, :, 1], op0=ALU.mult, op1=ALU.subtract)
                (dve if diag_dve else gp).tensor_add(
                    ud[:, :, 1], sqe[:, :, 0], sqe[:, :, 1])
                rud = sg.tile([P, fc, 2], F32, tag="rud")
                dve.reciprocal_approx_fast(out=rud[:], in_=ud[:])

                scr = sg.tile([P, fc, 2], F32, tag="scr")
                dve.scalar_tensor_tensor(
                    out=scr[:], in0=ic[:], scalar=1.0, in1=rud[:],
                    op0=ALU.mult, op1=ALU.mult, accum_out=acc[:, i:i + 1],
                )

            for i in range(nch + lag):
                if i < nch:
                    front(i)
                if i >= lag:
                    back(i - lag)

            nc.sync.dma_start(out=acc_d[:], in_=acc[:])

    nc.finalize()
    return nc


def _shard(arr):
    """arr [N_BOXES, 4] -> list of 8 per-core [P, W, 4] arrays (tail padded)."""
    v = np.ascontiguousarray(arr, dtype=np.float32).reshape(P, COLS, 4)
    pad_cols = N_CORES * W - COLS
    dummy = np.tile(
        np.array([0.0, 0.0, 1.0, 1.0], dtype=np.float32), (P, pad_cols, 1)
    )
    full = np.concatenate([v, dummy], axis=1)
    return [np.ascontiguousarray(full[:, c * W:(c + 1) * W, :]) for c in range(N_CORES)]


def kernel(pred_boxes, target_boxes):
    if "nc" not in _CACHE:
        _CACHE["nc"] = _build_program(chunks=CHUNKS, bwk=3, tail_gp=1)
        _CACHE["nch"] = len(CHUNKS)
    nc = _CACHE["nc"]

    preds = _shard(np.asarray(pred_boxes))
    targs = _shard(np.asarray(target_boxes))
    in_maps = [{"pred": preds[c], "targ": targs[c]} for c in range(N_CORES)]

    # the device occasionally reports a transient NRT_EXEC_UNIT_UNRECOVERABLE
    # wedge; it clears on re-execution, so retry a few times
    last_err = None
    for _attempt in range(4):
        try:
            res = run_bass_kernel_spmd(nc, in_maps, list(range(N_CORES)))
            break
        except Exception as e:
            last_err = e
    else:
        raise last_err

    # each acc column already holds sum(iou - cd/diag) for one chunk
    s = 0.0
    for c in range(N_CORES):
        s += res.results[c]["acc"].astype(np.float64).sum()
    # padded identity boxes contribute iou-ratio = 1 each
    s -= float(PAD_BOXES)
    loss = 1.0 - s / float(N_BOXES)
    return np.float32(loss)



# revision 2
# speedup vs baseline: 1.0245x; 1.0245x over previous
"""DIoU loss (mean) on 8 TRN2 cores — fp16 planar, HW-legal op set.

Host: scale boxes by 0.25 (loss is scale-invariant), cast fp16, per-core
DRAM laid out as plane-major chunk slabs (p1x|p1y|p2x|p2y planes per
chunk, contiguous per partition). fp16 halves DMA and unlocks the DVE
2x tensor_tensor mode.

Per chunk (z = p - t; planes x,y):
  ap = p23-p01, at = t23-t01, g = ap+at          [DVE TT]
  absz = |z| [ACT Abs], h = absz01+absz23        [DVE TT]
  u2 = g-h, e2 = g+h                             [DVE TT]
  dct = z01+z23                                  [Pool TT]
  sq = Square(dct||e2)                           [ACT, merged tile]
  ru = relu(u2) [ACT], inter = ru_x*ru_y         [DVE TT]
  areap [DVE], areat [Pool], asum [Pool]
  union4 = 4*asum - inter                        [DVE stt, f32]
  diag4 = sq_e2x+sq_e2y [Pool TT f32]; cd4 = sq_dx+sq_dy [Pool TT fp16]
  rud = recip(union4||diag4)                     [DVE custom]
  accA[:,i] += inter*rud_u; accB[:,i] += cd4*rud_d   [DVE stt accum]
Host: loss = 1 - (sum(accA) - sum(accB) - PAD)/N.
"""

import numpy as np

import concourse.bass as bass
import concourse.mybir as mybir
from concourse import bacc
from concourse.tile import TileContext
from concourse.bass_utils import run_bass_kernel_spmd

N_BOXES = 2_000_000
P = 128
COLS = N_BOXES // P            # 15625
N_CORES = 8
W = 1956                       # cols per core (8*1956 = 15648 >= 15625)
PAD_BOXES = N_CORES * W * P - N_BOXES  # 2944

F32 = mybir.dt.float32
FP16 = mybir.dt.float16
ALU = mybir.AluOpType
AF = mybir.ActivationFunctionType

_CACHE = {}

# engine assignment: 'dve' (tensor_tensor) | 'pool' (gpsimd tensor_tensor)
# plus special values documented per-op
DEFAULT_ASSIGN = dict(
    z="dve", ap="dve", at="dve", g="dve",
    absz="act",          # ACT Abs (DVE ts abs_max is illegal on HW)
    h="dve",
    u2="dve", e2="dve",
    dct="pool",
    sq="act",
    ru="act",            # ACT Relu | 'dve' ts max | 'pool' ts max
    inter="dve",
    areap="dve", areat="pool", asum="pool",
    diag="pool", cd4="pool", union="pool",
    final="dve",
)


def _build_program(chunks=None, nch=6, assign=None, bio=2, bwk=2, bsg=2,
                   lag=1, lag2=2):
    a = dict(DEFAULT_ASSIGN)
    if assign:
        a.update(assign)
    if chunks is None:
        fc0 = W // nch
        chunks = [fc0] * nch
        chunks[-1] = W - fc0 * (nch - 1)
    nch = len(chunks)
    assert sum(chunks) == W
    offs = [4 * sum(chunks[:i]) for i in range(nch)]
    nc = bacc.Bacc(None, target_bir_lowering=False)

    pred_d = nc.dram_tensor("pred", [P, 4 * W], FP16, kind="ExternalInput")
    targ_d = nc.dram_tensor("targ", [P, 4 * W], FP16, kind="ExternalInput")
    acc_d = nc.dram_tensor("acc", [P, 2 * nch], F32, kind="ExternalOutput")

    dve = nc.vector
    gp = nc.gpsimd
    act = nc.scalar

    def tt(name, out, in0, in1, op):
        e = a[name]
        if e == "dve":
            dve.tensor_tensor(out=out, in0=in0, in1=in1, op=op)
        else:
            gp.tensor_tensor(out=out, in0=in0, in1=in1, op=op)

    with TileContext(nc) as tc:
        with (
            tc.tile_pool(name="io", bufs=bio) as io,
            tc.tile_pool(name="wk", bufs=bwk) as wk,
            tc.tile_pool(name="sg", bufs=bsg) as sg,
            tc.tile_pool(name="accp", bufs=1) as accp,
        ):
            acc = accp.tile([P, 2 * nch], F32)
            state = {}

            def stage_a(i):
                fc = chunks[i]
                o = offs[i]
                pt = io.tile([P, 4 * fc], FP16, tag="pred")
                tg = io.tile([P, 4 * fc], FP16, tag="targ")
                z = wk.tile([P, 4 * fc], FP16, tag="z")
                if i == 0 and a.get("split0", False):
                    # halve the first loads so z starts ~900ns earlier
                    nc.sync.dma_start(out=pt[:, :2 * fc],
                                      in_=pred_d[:, o:o + 2 * fc])
                    nc.sync.dma_start(out=tg[:, :2 * fc],
                                      in_=targ_d[:, o:o + 2 * fc])
                    nc.sync.dma_start(out=pt[:, 2 * fc:],
                                      in_=pred_d[:, o + 2 * fc:o + 4 * fc])
                    nc.sync.dma_start(out=tg[:, 2 * fc:],
                                      in_=targ_d[:, o + 2 * fc:o + 4 * fc])
                    tt("z", z[:, :2 * fc], pt[:, :2 * fc], tg[:, :2 * fc],
                       ALU.subtract)
                    tt("z", z[:, 2 * fc:], pt[:, 2 * fc:], tg[:, 2 * fc:],
                       ALU.subtract)
                else:
                    nc.sync.dma_start(out=pt[:], in_=pred_d[:, o:o + 4 * fc])
                    nc.sync.dma_start(out=tg[:], in_=targ_d[:, o:o + 4 * fc])
                    tt("z", z[:], pt[:], tg[:], ALU.subtract)
                ap = wk.tile([P, 2 * fc], FP16, tag="ap")
                tt("ap", ap[:], pt[:, 2 * fc:], pt[:, :2 * fc], ALU.subtract)
                at = wk.tile([P, 2 * fc], FP16, tag="at")
                tt("at", at[:], tg[:, 2 * fc:], tg[:, :2 * fc], ALU.subtract)
                g = wk.tile([P, 2 * fc], FP16, tag="g")
                tt("g", g[:], ap[:], at[:], ALU.add)

                absz = wk.tile([P, 4 * fc], FP16, tag="absz")
                act.activation(absz[:], z[:], AF.Abs)
                state[i] = [pt, tg, z, ap, at, g, absz]

            def stage_b(i):
                fc = chunks[i]
                pt, tg, z, ap, at, g, absz = state[i]
                h = wk.tile([P, 2 * fc], FP16, tag="h")
                tt("h", h[:], absz[:, :2 * fc], absz[:, 2 * fc:], ALU.add)

                u2 = wk.tile([P, 2 * fc], FP16, tag="u2")
                tt("u2", u2[:], g[:], h[:], ALU.subtract)

                de = wk.tile([P, 4 * fc], FP16, tag="de")
                tt("dct", de[:, :2 * fc], z[:, :2 * fc], z[:, 2 * fc:], ALU.add)
                tt("e2", de[:, 2 * fc:], g[:], h[:], ALU.add)

                sq = wk.tile([P, 4 * fc], FP16, tag="sq")
                act.activation(sq[:], de[:], AF.Square)
                state[i] = [ap, at, u2, sq]

            def back(i):
                fc = chunks[i]
                last = (i == nch - 1) and a.get("last_dve", False)

                def ttl(name, out, in0, in1, op):
                    if last:
                        dve.tensor_tensor(out=out, in0=in0, in1=in1, op=op)
                    else:
                        tt(name, out, in0, in1, op)
                ap, at, u2, sq = state.pop(i)
                ru = sg.tile([P, 2 * fc], FP16, tag="ru")
                # relu(0.5*u2) = true overlap, so union = asum - inter is a
                # plain TT sub (no stt needed)
                act.activation(ru[:], u2[:], AF.Relu, scale=0.5)
                inter = sg.tile([P, fc], FP16, tag="inter")
                tt("inter", inter[:], ru[:, :fc], ru[:, fc:], ALU.mult)

                areap = sg.tile([P, fc], FP16, tag="areap")
                ttl("areap", areap[:], ap[:, :fc], ap[:, fc:], ALU.mult)
                areat = sg.tile([P, fc], FP16, tag="areat")
                ttl("areat", areat[:], at[:, :fc], at[:, fc:], ALU.mult)
                asum = sg.tile([P, fc], FP16, tag="asum")
                ttl("asum", asum[:], areap[:], areat[:], ALU.add)

                ud = sg.tile([P, 2 * fc], F32, tag="ud")
                tt("union", ud[:, :fc], asum[:], inter[:], ALU.subtract)
                ttl("diag", ud[:, fc:], sq[:, 2 * fc:3 * fc], sq[:, 3 * fc:],
                   ALU.add)
                cd4 = sg.tile([P, fc], FP16, tag="cd4")
                ttl("cd4", cd4[:], sq[:, :fc], sq[:, fc:2 * fc], ALU.add)

                rud = sg.tile([P, 2 * fc], F32, tag="rud")
                dve.reciprocal_approx_fast(out=rud[:], in_=ud[:])

                scrA = sg.tile([P, fc], F32, tag="scrA")
                dve.scalar_tensor_tensor(
                    out=scrA[:], in0=inter[:], scalar=1.0, in1=rud[:, :fc],
                    op0=ALU.mult, op1=ALU.mult, accum_out=acc[:, i:i + 1])
                scrB = sg.tile([P, fc], F32, tag="scrB")
                dve.scalar_tensor_tensor(
                    out=scrB[:], in0=cd4[:], scalar=1.0, in1=rud[:, fc:],
                    op0=ALU.mult, op1=ALU.mult,
                    accum_out=acc[:, nch + i:nch + i + 1])

            order = a.get("order", "abk")
            for i in range(nch + lag + lag2):
                def do_a():
                    if i < nch:
                        stage_a(i)
                def do_b():
                    if lag <= i < nch + lag:
                        stage_b(i - lag)
                def do_k():
                    if i >= lag + lag2:
                        back(i - lag - lag2)
                steps = {"a": do_a, "b": do_b, "k": do_k}
                for s in order:
                    steps[s]()

            nc.sync.dma_start(out=acc_d[:], in_=acc[:])

    nc.finalize()
    return nc


def _shard(arr, chunks):
    """[N,4] f32 -> per-core [P, 4W] fp16 plane-slab layout (scaled 0.25)."""
    v = (np.asarray(arr, dtype=np.float32) * 0.25).astype(np.float16)
    v = v.reshape(P, COLS, 4)
    pad_cols = N_CORES * W - COLS
    dummy = np.tile(np.array([0.0, 0.0, 0.25, 0.25], dtype=np.float16),
                    (P, pad_cols, 1))
    full = np.concatenate([v, dummy], axis=1)        # [P, 8W, 4]
    out = []
    for c in range(N_CORES):
        cv = full[:, c * W:(c + 1) * W, :]           # [P, W, 4]
        slabs = []
        off = 0
        for fc in chunks:
            s = cv[:, off:off + fc, :].transpose(0, 2, 1)  # [P,4,fc]
            slabs.append(s.reshape(P, 4 * fc))
            off += fc
        out.append(np.ascontiguousarray(np.concatenate(slabs, axis=1)))
    return out


BEST = dict(nch=6, lag=1, lag2=1, assign=dict(union="dve", areap="pool"))


def kernel(pred_boxes, target_boxes):
    if "nc" not in _CACHE:
        _CACHE["nc"] = _build_program(**BEST)
        ch = BEST.get("chunks")
        if ch is None:
            nch = BEST["nch"]
            fc0 = W // nch
            ch = [fc0] * nch
            ch[-1] = W - fc0 * (nch - 1)
        _CACHE["chunks"] = ch
        _CACHE["nch"] = len(ch)
    nc = _CACHE["nc"]
    nch = _CACHE["nch"]

    preds = _shard(np.asarray(pred_boxes), _CACHE["chunks"])
    targs = _shard(np.asarray(target_boxes), _CACHE["chunks"])
    in_maps = [{"pred": preds[c], "targ": targs[c]} for c in range(N_CORES)]

    last_err = None
    for _attempt in range(4):
        try:
            res = run_bass_kernel_spmd(nc, in_maps, list(range(N_CORES)))
            break
        except Exception as e:
            last_err = e
    else:
        raise last_err

    sA = 0.0
    sB = 0.0
    for c in range(N_CORES):
        av = res.results[c]["acc"].astype(np.float64)
        sA += av[:, :nch].sum()
        sB += av[:, nch:].sum()
    s = sA - sB - float(PAD_BOXES)
    loss = 1.0 - s / float(N_BOXES)
    return np.float32(loss)


# revision 3
# speedup vs baseline: 1.0931x; 1.0670x over previous
"""DIoU loss (mean) on 8 TRN2 cores — fp16 planar, HW-legal op set.

Host: scale boxes by 0.25 (loss is scale-invariant), cast fp16, per-core
DRAM laid out as plane-major chunk slabs (p1x|p1y|p2x|p2y planes per
chunk, contiguous per partition). fp16 halves DMA and unlocks the DVE
2x tensor_tensor mode.

Per chunk (z = p - t; planes x,y):
  ap = p23-p01, at = t23-t01, g = ap+at          [DVE TT]
  absz = |z| [ACT Abs], h = absz01+absz23        [DVE TT]
  u2 = g-h, e2 = g+h                             [DVE TT]
  dct = z01+z23                                  [Pool TT]
  sq = Square(dct||e2)                           [ACT, merged tile]
  ru = relu(u2) [ACT], inter = ru_x*ru_y         [DVE TT]
  areap [DVE], areat [Pool], asum [Pool]
  union4 = 4*asum - inter                        [DVE stt, f32]
  diag4 = sq_e2x+sq_e2y [Pool TT f32]; cd4 = sq_dx+sq_dy [Pool TT fp16]
  rud = recip(union4||diag4)                     [DVE custom]
  accA[:,i] += inter*rud_u; accB[:,i] += cd4*rud_d   [DVE stt accum]
Host: loss = 1 - (sum(accA) - sum(accB) - PAD)/N.
"""

import numpy as np

import concourse.bass as bass
import concourse.mybir as mybir
from concourse import bacc
from concourse.tile import TileContext
from concourse.bass_utils import run_bass_kernel_spmd

N_BOXES = 2_000_000
P = 128
COLS = N_BOXES // P            # 15625
N_CORES = 8
W = 1956                       # cols per core (8*1956 = 15648 >= 15625)
PAD_BOXES = N_CORES * W * P - N_BOXES  # 2944

F32 = mybir.dt.float32
FP16 = mybir.dt.float16
ALU = mybir.AluOpType
AF = mybir.ActivationFunctionType

_CACHE = {}

# engine assignment: 'dve' (tensor_tensor) | 'pool' (gpsimd tensor_tensor)
# plus special values documented per-op
DEFAULT_ASSIGN = dict(
    z="dve", ap="dve", at="dve", g="dve",
    absz="act",          # ACT Abs (DVE ts abs_max is illegal on HW)
    h="dve",
    u2="dve", e2="dve",
    dct="pool",
    sq="act",
    ru="act",            # ACT Relu | 'dve' ts max | 'pool' ts max
    inter="dve",
    areap="dve", areat="pool", asum="pool",
    diag="pool", cd4="pool", union="pool",
    final="dve",
)


def _build_program(chunks=None, nch=6, assign=None, bio=2, bwk=2, bsg=2,
                   lag=1, lag2=2):
    a = dict(DEFAULT_ASSIGN)
    if assign:
        a.update(assign)
    if chunks is None:
        fc0 = W // nch
        chunks = [fc0] * nch
        chunks[-1] = W - fc0 * (nch - 1)
    nch = len(chunks)
    assert sum(chunks) == W
    offs = [4 * sum(chunks[:i]) for i in range(nch)]
    nc = bacc.Bacc(None, target_bir_lowering=False)

    pred_d = nc.dram_tensor("pred", [P, 4 * W], FP16, kind="ExternalInput")
    targ_d = nc.dram_tensor("targ", [P, 4 * W], FP16, kind="ExternalInput")
    acc_d = nc.dram_tensor("acc", [P, 2 * nch], F32, kind="ExternalOutput")

    dve = nc.vector
    gp = nc.gpsimd
    act = nc.scalar

    def tt(name, out, in0, in1, op):
        e = a[name]
        if e == "dve":
            dve.tensor_tensor(out=out, in0=in0, in1=in1, op=op)
        else:
            gp.tensor_tensor(out=out, in0=in0, in1=in1, op=op)

    with TileContext(nc) as tc:
        with (
            tc.tile_pool(name="io", bufs=bio) as io,
            tc.tile_pool(name="wk", bufs=bwk) as wk,
            tc.tile_pool(name="sg", bufs=bsg) as sg,
            tc.tile_pool(name="accp", bufs=1) as accp,
        ):
            acc = accp.tile([P, 2 * nch], F32)
            state = {}

            def stage_a(i):
                fc = chunks[i]
                o = offs[i]
                pt = io.tile([P, 4 * fc], FP16, tag="pred")
                tg = io.tile([P, 4 * fc], FP16, tag="targ")
                z = wk.tile([P, 4 * fc], FP16, tag="z")
                if i == 0 and a.get("split0", False):
                    # halve the first loads so z starts ~900ns earlier
                    nc.sync.dma_start(out=pt[:, :2 * fc],
                                      in_=pred_d[:, o:o + 2 * fc])
                    nc.sync.dma_start(out=tg[:, :2 * fc],
                                      in_=targ_d[:, o:o + 2 * fc])
                    nc.sync.dma_start(out=pt[:, 2 * fc:],
                                      in_=pred_d[:, o + 2 * fc:o + 4 * fc])
                    nc.sync.dma_start(out=tg[:, 2 * fc:],
                                      in_=targ_d[:, o + 2 * fc:o + 4 * fc])
                    tt("z", z[:, :2 * fc], pt[:, :2 * fc], tg[:, :2 * fc],
                       ALU.subtract)
                    tt("z", z[:, 2 * fc:], pt[:, 2 * fc:], tg[:, 2 * fc:],
                       ALU.subtract)
                else:
                    nc.sync.dma_start(out=pt[:], in_=pred_d[:, o:o + 4 * fc])
                    nc.sync.dma_start(out=tg[:], in_=targ_d[:, o:o + 4 * fc])
                    tt("z", z[:], pt[:], tg[:], ALU.subtract)
                ap = wk.tile([P, 2 * fc], FP16, tag="ap")
                tt("ap", ap[:], pt[:, 2 * fc:], pt[:, :2 * fc], ALU.subtract)
                at = wk.tile([P, 2 * fc], FP16, tag="at")
                tt("at", at[:], tg[:, 2 * fc:], tg[:, :2 * fc], ALU.subtract)
                g = wk.tile([P, 2 * fc], FP16, tag="g")
                tt("g", g[:], ap[:], at[:], ALU.add)

                absz = wk.tile([P, 4 * fc], FP16, tag="absz")
                act.activation(absz[:], z[:], AF.Abs)
                state[i] = [pt, tg, z, ap, at, g, absz]

            def stage_b(i):
                fc = chunks[i]
                pt, tg, z, ap, at, g, absz = state[i]
                h = wk.tile([P, 2 * fc], FP16, tag="h")
                tt("h", h[:], absz[:, :2 * fc], absz[:, 2 * fc:], ALU.add)

                u2 = wk.tile([P, 2 * fc], FP16, tag="u2")
                tt("u2", u2[:], g[:], h[:], ALU.subtract)

                de = wk.tile([P, 4 * fc], FP16, tag="de")
                tt("dct", de[:, :2 * fc], z[:, :2 * fc], z[:, 2 * fc:], ALU.add)
                tt("e2", de[:, 2 * fc:], g[:], h[:], ALU.add)

                sq = wk.tile([P, 4 * fc], FP16, tag="sq")
                act.activation(sq[:], de[:], AF.Square)

                if a.get("early", False):
                    ru = wk.tile([P, 2 * fc], FP16, tag="ru")
                    act.activation(ru[:], u2[:], AF.Relu, scale=0.5)
                    inter = wk.tile([P, fc], FP16, tag="inter")
                    tt("inter", inter[:], ru[:, :fc], ru[:, fc:], ALU.mult)
                    areap = wk.tile([P, fc], FP16, tag="areap")
                    tt("areap", areap[:], ap[:, :fc], ap[:, fc:], ALU.mult)
                    areat = wk.tile([P, fc], FP16, tag="areat")
                    tt("areat", areat[:], at[:, :fc], at[:, fc:], ALU.mult)
                    asum = wk.tile([P, fc], FP16, tag="asum")
                    tt("asum", asum[:], areap[:], areat[:], ALU.add)
                    ud = wk.tile([P, 2 * fc], F32, tag="ud")
                    tt("union", ud[:, :fc], asum[:], inter[:], ALU.subtract)
                    state[i] = [inter, sq, ud]
                else:
                    state[i] = [ap, at, u2, sq]

            def back(i):
                fc = chunks[i]
                last = (i == nch - 1) and a.get("last_dve", False)

                def ttl(name, out, in0, in1, op):
                    if last:
                        dve.tensor_tensor(out=out, in0=in0, in1=in1, op=op)
                    else:
                        tt(name, out, in0, in1, op)
                if a.get("early", False):
                    inter, sq, ud = state.pop(i)
                else:
                    ap, at, u2, sq = state.pop(i)
                    ru = sg.tile([P, 2 * fc], FP16, tag="ru")
                    # relu(0.5*u2) = true overlap, so union = asum - inter
                    # is a plain TT sub (no stt needed)
                    act.activation(ru[:], u2[:], AF.Relu, scale=0.5)
                    inter = sg.tile([P, fc], FP16, tag="inter")
                    tt("inter", inter[:], ru[:, :fc], ru[:, fc:], ALU.mult)

                    areap = sg.tile([P, fc], FP16, tag="areap")
                    ttl("areap", areap[:], ap[:, :fc], ap[:, fc:], ALU.mult)
                    areat = sg.tile([P, fc], FP16, tag="areat")
                    ttl("areat", areat[:], at[:, :fc], at[:, fc:], ALU.mult)
                    asum = sg.tile([P, fc], FP16, tag="asum")
                    ttl("asum", asum[:], areap[:], areat[:], ALU.add)

                    ud = sg.tile([P, 2 * fc], F32, tag="ud")
                    tt("union", ud[:, :fc], asum[:], inter[:], ALU.subtract)
                ttl("diag", ud[:, fc:], sq[:, 2 * fc:3 * fc], sq[:, 3 * fc:],
                   ALU.add)
                cd4 = sg.tile([P, fc], FP16, tag="cd4")
                ttl("cd4", cd4[:], sq[:, :fc], sq[:, fc:2 * fc], ALU.add)

                rud = sg.tile([P, 2 * fc], F32, tag="rud")
                dve.reciprocal_approx_fast(out=rud[:], in_=ud[:])

                scrA = sg.tile([P, fc], F32, tag="scrA")
                dve.scalar_tensor_tensor(
                    out=scrA[:], in0=inter[:], scalar=1.0, in1=rud[:, :fc],
                    op0=ALU.mult, op1=ALU.mult, accum_out=acc[:, i:i + 1])
                scrB = sg.tile([P, fc], F32, tag="scrB")
                dve.scalar_tensor_tensor(
                    out=scrB[:], in0=cd4[:], scalar=1.0, in1=rud[:, fc:],
                    op0=ALU.mult, op1=ALU.mult,
                    accum_out=acc[:, nch + i:nch + i + 1])

            order = a.get("order", "abk")
            for i in range(nch + lag + lag2):
                def do_a():
                    if i < nch:
                        stage_a(i)
                def do_b():
                    if lag <= i < nch + lag:
                        stage_b(i - lag)
                def do_k():
                    if i >= lag + lag2:
                        back(i - lag - lag2)
                steps = {"a": do_a, "b": do_b, "k": do_k}
                for s in order:
                    steps[s]()

            nc.sync.dma_start(out=acc_d[:], in_=acc[:])

    nc.finalize()
    return nc


def _shard(arr, chunks):
    """[N,4] f32 -> per-core [P, 4W] fp16 plane-slab layout (scaled 0.25)."""
    v = (np.asarray(arr, dtype=np.float32) * 0.25).astype(np.float16)
    v = v.reshape(P, COLS, 4)
    pad_cols = N_CORES * W - COLS
    dummy = np.tile(np.array([0.0, 0.0, 0.25, 0.25], dtype=np.float16),
                    (P, pad_cols, 1))
    full = np.concatenate([v, dummy], axis=1)        # [P, 8W, 4]
    out = []
    for c in range(N_CORES):
        cv = full[:, c * W:(c + 1) * W, :]           # [P, W, 4]
        slabs = []
        off = 0
        for fc in chunks:
            s = cv[:, off:off + fc, :].transpose(0, 2, 1)  # [P,4,fc]
            slabs.append(s.reshape(P, 4 * fc))
            off += fc
        out.append(np.ascontiguousarray(np.concatenate(slabs, axis=1)))
    return out


BEST = dict(chunks=[230, 615, 615, 496], lag=1, lag2=2, assign=dict(
    areap="pool", areat="pool", asum="pool", cd4="dve", dct="dve",
    diag="pool", inter="pool", union="pool"))


def kernel(pred_boxes, target_boxes):
    if "nc" not in _CACHE:
        _CACHE["nc"] = _build_program(**BEST)
        ch = BEST.get("chunks")
        if ch is None:
            nch = BEST["nch"]
            fc0 = W // nch
            ch = [fc0] * nch
            ch[-1] = W - fc0 * (nch - 1)
        _CACHE["chunks"] = ch
        _CACHE["nch"] = len(ch)
    nc = _CACHE["nc"]
    nch = _CACHE["nch"]

    preds = _shard(np.asarray(pred_boxes), _CACHE["chunks"])
    targs = _shard(np.asarray(target_boxes), _CACHE["chunks"])
    in_maps = [{"pred": preds[c], "targ": targs[c]} for c in range(N_CORES)]

    last_err = None
    for _attempt in range(4):
        try:
            res = run_bass_kernel_spmd(nc, in_maps, list(range(N_CORES)))
            break
        except Exception as e:
            last_err = e
    else:
        raise last_err

    sA = 0.0
    sB = 0.0
    for c in range(N_CORES):
        av = res.results[c]["acc"].astype(np.float64)
        sA += av[:, :nch].sum()
        sB += av[:, nch:].sum()
    s = sA - sB - float(PAD_BOXES)
    loss = 1.0 - s / float(N_BOXES)
    return np.float32(loss)


# revision 4
# speedup vs baseline: 1.0932x; 1.0001x over previous
"""DIoU loss (mean) on 8 TRN2 cores — fp16 planar, HW-legal op set.

Host: scale boxes by 0.25 (loss is scale-invariant), cast fp16, per-core
DRAM laid out as plane-major chunk slabs (p1x|p1y|p2x|p2y planes per
chunk, contiguous per partition). fp16 halves DMA and unlocks the DVE
2x tensor_tensor mode.

Per chunk (z = p - t; planes x,y):
  ap = p23-p01, at = t23-t01, g = ap+at          [DVE TT]
  absz = |z| [ACT Abs], h = absz01+absz23        [DVE TT]
  u2 = g-h, e2 = g+h                             [DVE TT]
  dct = z01+z23                                  [Pool TT]
  sq = Square(dct||e2)                           [ACT, merged tile]
  ru = relu(u2) [ACT], inter = ru_x*ru_y         [DVE TT]
  areap [DVE], areat [Pool], asum [Pool]
  union4 = 4*asum - inter                        [DVE stt, f32]
  diag4 = sq_e2x+sq_e2y [Pool TT f32]; cd4 = sq_dx+sq_dy [Pool TT fp16]
  rud = recip(union4||diag4)                     [DVE custom]
  accA[:,i] += inter*rud_u; accB[:,i] += cd4*rud_d   [DVE stt accum]
Host: loss = 1 - (sum(accA) - sum(accB) - PAD)/N.
"""

import numpy as np

import concourse.bass as bass
import concourse.mybir as mybir
from concourse import bacc
from concourse.tile import TileContext
from concourse.bass_utils import run_bass_kernel_spmd

N_BOXES = 2_000_000
P = 128
COLS = N_BOXES // P            # 15625
N_CORES = 8
W = 1956                       # cols per core (8*1956 = 15648 >= 15625)
PAD_BOXES = N_CORES * W * P - N_BOXES  # 2944

F32 = mybir.dt.float32
FP16 = mybir.dt.float16
ALU = mybir.AluOpType
AF = mybir.ActivationFunctionType

_CACHE = {}

# engine assignment: 'dve' (tensor_tensor) | 'pool' (gpsimd tensor_tensor)
# plus special values documented per-op
DEFAULT_ASSIGN = dict(
    z="dve", ap="dve", at="dve", g="dve",
    absz="act",          # ACT Abs (DVE ts abs_max is illegal on HW)
    h="dve",
    u2="dve", e2="dve",
    dct="pool",
    sq="act",
    ru="act",            # ACT Relu | 'dve' ts max | 'pool' ts max
    inter="dve",
    areap="dve", areat="pool", asum="pool",
    diag="pool", cd4="pool", union="pool",
    final="dve",
)


def _build_program(chunks=None, nch=6, assign=None, bio=2, bwk=2, bsg=2,
                   lag=1, lag2=2):
    a = dict(DEFAULT_ASSIGN)
    if assign:
        a.update(assign)
    if chunks is None:
        fc0 = W // nch
        chunks = [fc0] * nch
        chunks[-1] = W - fc0 * (nch - 1)
    nch = len(chunks)
    assert sum(chunks) == W
    offs = [4 * sum(chunks[:i]) for i in range(nch)]
    nc = bacc.Bacc(None, target_bir_lowering=False)

    pred_d = nc.dram_tensor("pred", [P, 4 * W], FP16, kind="ExternalInput")
    targ_d = nc.dram_tensor("targ", [P, 4 * W], FP16, kind="ExternalInput")
    acc_d = nc.dram_tensor("acc", [P, 2 * nch], F32, kind="ExternalOutput")

    dve = nc.vector
    gp = nc.gpsimd
    act = nc.scalar

    def tt(name, out, in0, in1, op):
        e = a[name]
        if e == "dve":
            dve.tensor_tensor(out=out, in0=in0, in1=in1, op=op)
        else:
            gp.tensor_tensor(out=out, in0=in0, in1=in1, op=op)

    with TileContext(nc) as tc:
        with (
            tc.tile_pool(name="io", bufs=bio) as io,
            tc.tile_pool(name="wk", bufs=bwk) as wk,
            tc.tile_pool(name="sg", bufs=bsg) as sg,
            tc.tile_pool(name="accp", bufs=1) as accp,
        ):
            acc = accp.tile([P, 2 * nch], F32)
            state = {}

            def stage_a(i):
                fc = chunks[i]
                o = offs[i]
                pt = io.tile([P, 4 * fc], FP16, tag="pred")
                tg = io.tile([P, 4 * fc], FP16, tag="targ")
                z = wk.tile([P, 4 * fc], FP16, tag="z")
                if i == 0 and a.get("split0", False):
                    # halve the first loads so z starts ~900ns earlier
                    nc.sync.dma_start(out=pt[:, :2 * fc],
                                      in_=pred_d[:, o:o + 2 * fc])
                    nc.sync.dma_start(out=tg[:, :2 * fc],
                                      in_=targ_d[:, o:o + 2 * fc])
                    nc.sync.dma_start(out=pt[:, 2 * fc:],
                                      in_=pred_d[:, o + 2 * fc:o + 4 * fc])
                    nc.sync.dma_start(out=tg[:, 2 * fc:],
                                      in_=targ_d[:, o + 2 * fc:o + 4 * fc])
                    tt("z", z[:, :2 * fc], pt[:, :2 * fc], tg[:, :2 * fc],
                       ALU.subtract)
                    tt("z", z[:, 2 * fc:], pt[:, 2 * fc:], tg[:, 2 * fc:],
                       ALU.subtract)
                else:
                    nc.sync.dma_start(out=pt[:], in_=pred_d[:, o:o + 4 * fc])
                    nc.sync.dma_start(out=tg[:], in_=targ_d[:, o:o + 4 * fc])
                    tt("z", z[:], pt[:], tg[:], ALU.subtract)
                ap = wk.tile([P, 2 * fc], FP16, tag="ap")
                tt("ap", ap[:], pt[:, 2 * fc:], pt[:, :2 * fc], ALU.subtract)
                at = wk.tile([P, 2 * fc], FP16, tag="at")
                tt("at", at[:], tg[:, 2 * fc:], tg[:, :2 * fc], ALU.subtract)
                g = wk.tile([P, 2 * fc], FP16, tag="g")
                tt("g", g[:], ap[:], at[:], ALU.add)

                absz = wk.tile([P, 4 * fc], FP16, tag="absz")
                act.activation(absz[:], z[:], AF.Abs)
                state[i] = [pt, tg, z, ap, at, g, absz]

            def stage_b(i):
                fc = chunks[i]
                pt, tg, z, ap, at, g, absz = state[i]
                h = wk.tile([P, 2 * fc], FP16, tag="h")
                tt("h", h[:], absz[:, :2 * fc], absz[:, 2 * fc:], ALU.add)

                u2 = wk.tile([P, 2 * fc], FP16, tag="u2")
                tt("u2", u2[:], g[:], h[:], ALU.subtract)

                de = wk.tile([P, 4 * fc], FP16, tag="de")
                tt("dct", de[:, :2 * fc], z[:, :2 * fc], z[:, 2 * fc:], ALU.add)
                tt("e2", de[:, 2 * fc:], g[:], h[:], ALU.add)

                sq = wk.tile([P, 4 * fc], FP16, tag="sq")
                act.activation(sq[:], de[:], AF.Square)

                if a.get("early", False):
                    ru = wk.tile([P, 2 * fc], FP16, tag="ru")
                    act.activation(ru[:], u2[:], AF.Relu, scale=0.5)
                    inter = wk.tile([P, fc], FP16, tag="inter")
                    tt("inter", inter[:], ru[:, :fc], ru[:, fc:], ALU.mult)
                    areap = wk.tile([P, fc], FP16, tag="areap")
                    tt("areap", areap[:], ap[:, :fc], ap[:, fc:], ALU.mult)
                    areat = wk.tile([P, fc], FP16, tag="areat")
                    tt("areat", areat[:], at[:, :fc], at[:, fc:], ALU.mult)
                    asum = wk.tile([P, fc], FP16, tag="asum")
                    tt("asum", asum[:], areap[:], areat[:], ALU.add)
                    ud = wk.tile([P, 2 * fc], F32, tag="ud")
                    tt("union", ud[:, :fc], asum[:], inter[:], ALU.subtract)
                    state[i] = [inter, sq, ud]
                else:
                    state[i] = [ap, at, u2, sq]

            def back(i):
                fc = chunks[i]
                last = (i == nch - 1) and a.get("last_dve", False)

                def ttl(name, out, in0, in1, op):
                    if last:
                        dve.tensor_tensor(out=out, in0=in0, in1=in1, op=op)
                    else:
                        tt(name, out, in0, in1, op)
                if a.get("early", False):
                    inter, sq, ud = state.pop(i)
                else:
                    ap, at, u2, sq = state.pop(i)
                    ru = sg.tile([P, 2 * fc], FP16, tag="ru")
                    # relu(0.5*u2) = true overlap, so union = asum - inter
                    # is a plain TT sub (no stt needed)
                    act.activation(ru[:], u2[:], AF.Relu, scale=0.5)
                    inter = sg.tile([P, fc], FP16, tag="inter")
                    tt("inter", inter[:], ru[:, :fc], ru[:, fc:], ALU.mult)

                    areap = sg.tile([P, fc], FP16, tag="areap")
                    ttl("areap", areap[:], ap[:, :fc], ap[:, fc:], ALU.mult)
                    areat = sg.tile([P, fc], FP16, tag="areat")
                    ttl("areat", areat[:], at[:, :fc], at[:, fc:], ALU.mult)
                    asum = sg.tile([P, fc], FP16, tag="asum")
                    ttl("asum", asum[:], areap[:], areat[:], ALU.add)

                    ud = sg.tile([P, 2 * fc], F32, tag="ud")
                    tt("union", ud[:, :fc], asum[:], inter[:], ALU.subtract)
                ttl("diag", ud[:, fc:], sq[:, 2 * fc:3 * fc], sq[:, 3 * fc:],
                   ALU.add)
                cd4 = sg.tile([P, fc], FP16, tag="cd4")
                ttl("cd4", cd4[:], sq[:, :fc], sq[:, fc:2 * fc], ALU.add)

                rud = sg.tile([P, 2 * fc], F32, tag="rud")
                dve.reciprocal_approx_fast(out=rud[:], in_=ud[:])

                scrA = sg.tile([P, fc], F32, tag="scrA")
                dve.scalar_tensor_tensor(
                    out=scrA[:], in0=inter[:], scalar=1.0, in1=rud[:, :fc],
                    op0=ALU.mult, op1=ALU.mult, accum_out=acc[:, i:i + 1])
                scrB = sg.tile([P, fc], F32, tag="scrB")
                dve.scalar_tensor_tensor(
                    out=scrB[:], in0=cd4[:], scalar=1.0, in1=rud[:, fc:],
                    op0=ALU.mult, op1=ALU.mult,
                    accum_out=acc[:, nch + i:nch + i + 1])

            order = a.get("order", "abk")
            for i in range(nch + lag + lag2):
                def do_a():
                    if i < nch:
                        stage_a(i)
                def do_b():
                    if lag <= i < nch + lag:
                        stage_b(i - lag)
                def do_k():
                    if i >= lag + lag2:
                        back(i - lag - lag2)
                steps = {"a": do_a, "b": do_b, "k": do_k}
                for s in order:
                    steps[s]()

            nc.sync.dma_start(out=acc_d[:], in_=acc[:])

    nc.finalize()
    return nc


def _shard(arr, chunks):
    """[N,4] f32 -> per-core [P, 4W] fp16 plane-slab layout (scaled 0.25)."""
    v = (np.asarray(arr, dtype=np.float32) * 0.25).astype(np.float16)
    v = v.reshape(P, COLS, 4)
    pad_cols = N_CORES * W - COLS
    dummy = np.tile(np.array([0.0, 0.0, 0.25, 0.25], dtype=np.float16),
                    (P, pad_cols, 1))
    full = np.concatenate([v, dummy], axis=1)        # [P, 8W, 4]
    out = []
    for c in range(N_CORES):
        cv = full[:, c * W:(c + 1) * W, :]           # [P, W, 4]
        slabs = []
        off = 0
        for fc in chunks:
            s = cv[:, off:off + fc, :].transpose(0, 2, 1)  # [P,4,fc]
            slabs.append(s.reshape(P, 4 * fc))
            off += fc
        out.append(np.ascontiguousarray(np.concatenate(slabs, axis=1)))
    return out


BEST = dict(chunks=[230, 615, 599, 512], lag=1, lag2=2, assign=dict(
    areap="pool", areat="pool", asum="pool", cd4="dve", dct="dve",
    diag="pool", inter="pool", union="pool"))


def kernel(pred_boxes, target_boxes):
    if "nc" not in _CACHE:
        _CACHE["nc"] = _build_program(**BEST)
        ch = BEST.get("chunks")
        if ch is None:
            nch = BEST["nch"]
            fc0 = W // nch
            ch = [fc0] * nch
            ch[-1] = W - fc0 * (nch - 1)
        _CACHE["chunks"] = ch
        _CACHE["nch"] = len(ch)
    nc = _CACHE["nc"]
    nch = _CACHE["nch"]

    preds = _shard(np.asarray(pred_boxes), _CACHE["chunks"])
    targs = _shard(np.asarray(target_boxes), _CACHE["chunks"])
    in_maps = [{"pred": preds[c], "targ": targs[c]} for c in range(N_CORES)]

    last_err = None
    for _attempt in range(4):
        try:
            res = run_bass_kernel_spmd(nc, in_maps, list(range(N_CORES)))
            break
        except Exception as e:
            last_err = e
    else:
        raise last_err

    sA = 0.0
    sB = 0.0
    for c in range(N_CORES):
        av = res.results[c]["acc"].astype(np.float64)
        sA += av[:, :nch].sum()
        sB += av[:, nch:].sum()
    s = sA - sB - float(PAD_BOXES)
    loss = 1.0 - s / float(N_BOXES)
    return np.float32(loss)
